# revision 5
# baseline (speedup 1.0000x reference)
"""DynamicCenterLoss on Trainium2 (Bass/Tile), 8-core SPMD — v3.

Strategy: `batch` is sorted, so core b owns batch b (~N/8 points).
The wire format is a per-point fp8-e4m3 encoding built on the host:

    ext[n] = [ feat_n (64) | 1 | ||feat_n||^2 / 16 ]   (66 bytes/point)

4.33 MB/core instead of 17 MB in f32 — the DMA roofline drops from
~47us to ~12us/core.  Loss tolerance is 2e-2; the fp8 quantization
costs 9.5e-4 (measured on the real inputs).

Every reduction runs on-device through one PE pass: the per-class
one-hot segment matmul  OUT[13, 66] = sum_n onehot(tgt_n)^T (x) ext_n
yields per-class feature sums (cols 0:64), counts (col 64) and
per-class sum ||f||^2 / 16 (col 65) in a single PSUM accumulation,
alternating 2 PE column groups so each chunk's LDWEIGHTS hides under
the other group's MATMUL.  The one-hot is built on the Vector engine
(fp8 is_equal against a replicated iota).  The tiny pairwise-center
hinge + final divisions run on the host from the 8x[13,66] stats.

All ext tile DMAs are issued up-front (whole fp8 shard = 34 KB per
SBUF partition) on both HWDGE rings (sync + scalar), so the 16 SDMA
engines run back-to-back with zero buffer-recycle stalls.
"""

import numpy as np
import ml_dtypes

import concourse.bass as bass
import concourse.bacc as bacc
import concourse.tile as tile
from concourse import mybir
from concourse.bass_utils import run_bass_kernel_spmd

P = 128
D = 64
DE = D + 2  # [feat | 1 | hsq]
C = 13
B = 8
N_CORES = 8
MARGIN = 0.5
INTRA_W = 1.0
INTER_W = 1.0
LOSS_W = 0.01
IGNORE = -1
TT = 64  # points per SBUF tile step
SQ_SCALE = 16.0  # hsq = ||f||^2 / SQ_SCALE (fits e4m3 nicely)

NGRP = 2  # PE column groups (PSUM quadrants)

f32 = mybir.dt.float32
f8 = mybir.dt.float8e4
i32 = mybir.dt.int32

NP_F8 = ml_dtypes.float8_e4m3


def _splits(T: int):
    splits = []
    t0 = 0
    szs = [16, 48]
    while sum(szs) + TT <= T:
        szs.append(TT)
    for sz in szs:
        if t0 >= T:
            break
        sz = min(sz, T - t0)
        splits.append((t0, sz))
        t0 += sz
    if t0 < T:
        splits.append((t0, T - t0))
    return splits


def build_nc(T: int) -> bass.Bass:
    Npad = P * T
    splits = _splits(T)

    nc = bacc.Bacc("TRN2", target_bir_lowering=False)
    ext_h = nc.dram_tensor("ext", [Npad, DE], f8, kind="ExternalInput")
    tgt_h = nc.dram_tensor("tgt", [Npad], f8, kind="ExternalInput")
    out_h = nc.dram_tensor("out", [32 * (NGRP - 1) + C, DE], f32,
                           kind="ExternalOutput")

    extv = ext_h[:, :].rearrange("(p t) d -> p t d", p=P)  # [128, T, 66]
    tgtv = tgt_h[:].rearrange("(p t) -> p t", p=P)  # [128, T]

    with tile.TileContext(nc) as tc:
        with (
            tc.tile_pool(name="consts", bufs=1) as cp,
            tc.tile_pool(name="io", bufs=1) as iop,
            tc.tile_pool(name="oh", bufs=1) as ohp,
            tc.tile_pool(name="acc", bufs=1, space="PSUM") as psa,
            tc.tile_pool(name="fin", bufs=1) as fp,
        ):
            # ---- constants ----
            iota32 = cp.tile([P, TT, C], i32)
            nc.gpsimd.iota(
                iota32[:, :, :], pattern=[[0, TT], [1, C]], base=0,
                channel_multiplier=0,
            )
            iota8 = cp.tile([P, TT, C], f8)
            nc.vector.tensor_copy(iota8[:, :, :], iota32[:, :, :])
            tgt_sb = cp.tile([P, T], f8)
            nc.scalar.dma_start(out=tgt_sb[:, :], in_=tgtv[:, :])

            # ---- all ext tile DMAs up-front, alternating HWDGE rings.
            # One SBUF tile per DMA: disjoint writes into a single big
            # tile get serialized by whole-tile WAW tracking (measured:
            # 24.8us DMA span at 24% engine duty), separate tiles don't.
            exts = []
            for i, (t0, tt) in enumerate(splits):
                e = iop.tile([P, tt, DE], f8, name=f"ext{i}", tag=f"ext{i}")
                eng = nc.sync if i % 2 == 0 else nc.scalar
                eng.dma_start(out=e[:, :, :], in_=extv[:, t0 : t0 + tt, :])
                exts.append(e)

            ohs = [
                ohp.tile([P, tt, C], f8, name=f"oh{i}", tag=f"oh{i}")
                for i, (t0, tt) in enumerate(splits)
            ]

            # PE accumulation bookkeeping
            accs = []
            for g in range(NGRP):
                a = psa.tile([32 * g + C, DE], f32, name=f"accq{g}")
                accs.append(a[32 * g : 32 * g + C, :])
            started = [False] * NGRP
            last_step = [-1] * NGRP
            for s in range(T):
                last_step[s % NGRP] = s

            step = 0
            for i, (t0, tt) in enumerate(splits):
                # one-hot for this tile (vector engine, all-fp8)
                nc.vector.tensor_tensor(
                    out=ohs[i][:, :, :],
                    in0=tgt_sb[:, t0 : t0 + tt].unsqueeze(2).to_broadcast(
                        [P, tt, C]
                    ),
                    in1=iota8[:, :tt, :],
                    op=mybir.AluOpType.is_equal,
                )
                # one-hot segment matmuls, alternating PE column groups
                for t in range(tt):
                    g = step % NGRP
                    nc.tensor.matmul(
                        accs[g],
                        lhsT=ohs[i][:, t, :],
                        rhs=exts[i][:, t, :],
                        start=not started[g],
                        stop=(step == last_step[g]),
                        tile_position=(0, 32 * g),
                    )
                    started[g] = True
                    step += 1

            # ---- tail: stats to SBUF, single small out DMA ----
            out_sb = fp.tile([32 * (NGRP - 1) + C, DE], f32)
            for g in range(NGRP):
                nc.vector.tensor_copy(
                    out_sb[32 * g : 32 * g + C, :], accs[g]
                )
            nc.sync.dma_start(out=out_h[:, :], in_=out_sb[:, :])
    nc.finalize()
    return nc


# set by test.py to capture profile info
TRACE = False
LAST = {}


def _ensure_ntff_hook():
    """The agent image's antenv lacks axon_hooks; synthesize it so
    run_bass_kernel_spmd(trace=True) can profile. Best-effort."""
    import sys
    import types

    try:
        from antenv.axon_hooks import get_axon_ntff_profile_hook  # noqa: F401
        return
    except ImportError:
        pass
    try:
        from trn_agent_boot.trn_boot import _ntff_profile_via_ctypes

        hook = _ntff_profile_via_ctypes("/opt/axon/libaxon_pjrt.so")
        mod = types.ModuleType("antenv.axon_hooks")
        mod._hook = hook
        mod.get_axon_ntff_profile_hook = lambda: mod._hook
        mod.set_axon_ntff_profile_hook = lambda h: setattr(mod, "_hook", h)
        sys.modules["antenv.axon_hooks"] = mod
        import antenv

        antenv.axon_hooks = mod
    except Exception as e:  # degrade: no profile, run still works
        print(f"ntff hook injection failed: {e}")


def kernel(pred=None, target=None, feat=None, batch=None, centers=None):
    target = np.asarray(target)
    feat = np.asarray(feat, dtype=np.float32)
    batch = np.asarray(batch)
    centers = np.asarray(centers, dtype=np.float64)
    N = feat.shape[0]

    # shard at batch boundaries: core b <- batch b (batch is sorted)
    bounds = np.searchsorted(batch, np.arange(B + 1))
    sizes = np.diff(bounds)
    T = int(max((int(sizes.max()) + P - 1) // P, TT))
    Npad = P * T

    feat8 = feat.astype(NP_F8)
    hsq8 = ((feat8.astype(np.float32) ** 2).sum(1) / SQ_SCALE).astype(NP_F8)
    in_maps = []
    for b in range(B):
        lo, hi = int(bounds[b]), int(bounds[b + 1])
        n = hi - lo
        ext = np.zeros((Npad, DE), dtype=NP_F8)
        ext[:n, :D] = feat8[lo:hi]
        ext[:n, D] = np.asarray(1.0, dtype=NP_F8)
        ext[:n, D + 1] = hsq8[lo:hi]
        tb = np.full((Npad,), C, dtype=np.float32)
        tb[:n] = target[lo:hi]
        inv = tb == IGNORE
        if inv.any():
            tb[inv] = C  # one-hot miss -> excluded everywhere
            ext[inv] = np.asarray(0.0, dtype=NP_F8)
        in_maps.append({"ext": ext, "tgt": tb.astype(NP_F8)})

    nc = build_nc(T)
    if TRACE:
        _ensure_ntff_hook()
    res = run_bass_kernel_spmd(nc, in_maps, list(range(N_CORES)), trace=TRACE)
    LAST["results"] = res

    # ---- host finale (tiny: 8 cores x [13, 66] stats) ----
    intra_sum = 0.0
    inter_sum = 0.0
    present_cnt = 0
    cn2 = (centers ** 2).sum(1)  # (13,)
    for b in range(B):
        o = np.asarray(res.results[b]["out"]).astype(np.float64)
        stats = o[0:C, :].copy()
        for g in range(1, NGRP):
            stats += o[32 * g : 32 * g + C, :]
        fsum = stats[:, :D]  # (13, 64)
        ccnt = stats[:, D]  # (13,)
        S = SQ_SCALE * stats[:, D + 1].sum()
        cnt_b = ccnt.sum()
        if cnt_b <= 0:
            continue
        present_cnt += 1
        # intra: S - 2 sum_c c.fsum + sum_c ccnt*||c||^2, / cnt
        tdot = float((centers * fsum).sum())
        utot = float((ccnt * cn2).sum())
        intra_sum += (S - 2.0 * tdot + utot) / cnt_b
        # inter: pairwise hinge on class means
        pres = ccnt > 0
        cm = fsum / np.maximum(ccnt, 1.0)[:, None]
        diff = cm[:, None, :] - cm[None, :, :]
        dd2 = (diff ** 2).sum(-1)
        eye = np.eye(C, dtype=bool)
        pm = pres[:, None] & pres[None, :] & ~eye
        dist = np.sqrt(np.where(pm, dd2, 1.0))
        terms = np.where(pm, np.maximum(MARGIN - dist, 0.0), 0.0)
        npairs = pm.sum()
        inter_sum += terms.sum() / max(npairs, 1)

    den = max(present_cnt, 1)
    loss = LOSS_W * (INTRA_W * intra_sum / den + INTER_W * inter_sum / den)
    return np.float32(loss)


# revision 11
# speedup vs baseline: 1.2218x; 1.2218x over previous
"""DynamicCenterLoss on Trainium2 (Bass/Tile), 8-core SPMD — v3.

Strategy: `batch` is sorted, so core b owns batch b (~N/8 points).
The wire format is a per-point fp8-e4m3 encoding built on the host:

    ext[n] = [ feat_n (64) | 1 | ||feat_n||^2 / 16 ]   (66 bytes/point)

4.33 MB/core instead of 17 MB in f32 — the DMA roofline drops from
~47us to ~12us/core.  Loss tolerance is 2e-2; the fp8 quantization
costs 9.5e-4 (measured on the real inputs).

Every reduction runs on-device through one PE pass: the per-class
one-hot segment matmul  OUT[13, 66] = sum_n onehot(tgt_n)^T (x) ext_n
yields per-class feature sums (cols 0:64), counts (col 64) and
per-class sum ||f||^2 / 16 (col 65) in a single PSUM accumulation,
alternating 2 PE column groups so each chunk's LDWEIGHTS hides under
the other group's MATMUL.  The one-hot is built on the Vector engine
(fp8 is_equal against a replicated iota).  The tiny pairwise-center
hinge + final divisions run on the host from the 8x[13,66] stats.

All ext tile DMAs are issued up-front (whole fp8 shard = 34 KB per
SBUF partition) on both HWDGE rings (sync + scalar), so the 16 SDMA
engines run back-to-back with zero buffer-recycle stalls.
"""

import numpy as np
import ml_dtypes

import concourse.bass as bass
import concourse.bacc as bacc
import concourse.tile as tile
from concourse import mybir
from concourse.bass_utils import run_bass_kernel_spmd

P = 128
D = 64
DE = D + 2  # [feat | 1 | hsq]
C = 13
B = 8
N_CORES = 8
MARGIN = 0.5
INTRA_W = 1.0
INTER_W = 1.0
LOSS_W = 0.01
IGNORE = -1
TT = 64  # points per SBUF tile step
C16 = 16  # one-hot padded to 16 classes: DoubleRow LDWEIGHTS needs 16B-aligned Ko-step
SQ_SCALE = 16.0  # hsq = ||f||^2 / SQ_SCALE (fits e4m3 nicely)

NGRP = 1  # DoubleRow is incompatible with PE column groups

f32 = mybir.dt.float32
f8 = mybir.dt.float8e4
i32 = mybir.dt.int32

NP_F8 = ml_dtypes.float8_e4m3


def _splits(T: int):
    assert T % 2 == 0
    splits = []
    t0 = 0
    szs = [16, 48]
    while sum(szs) + TT <= T:
        szs.append(TT)
    for sz in szs:
        if t0 >= T:
            break
        sz = min(sz, T - t0)
        splits.append((t0, sz))
        t0 += sz
    if t0 < T:
        splits.append((t0, T - t0))
    assert all(tt % 2 == 0 for _, tt in splits)
    return splits


def build_nc(T: int) -> bass.Bass:
    Npad = P * T
    splits = _splits(T)

    nc = bacc.Bacc("TRN2", target_bir_lowering=False)
    ext_h = nc.dram_tensor("ext", [Npad, DE], f8, kind="ExternalInput")
    tgt_h = nc.dram_tensor("tgt", [Npad], f8, kind="ExternalInput")
    out_h = nc.dram_tensor("out", [32 * (NGRP - 1) + C16, DE], f32,
                           kind="ExternalOutput")

    extv = ext_h[:, :].rearrange("(p t) d -> p t d", p=P)  # [128, T, 66]
    tgtv = tgt_h[:].rearrange("(p t) -> p t", p=P)  # [128, T]

    with tile.TileContext(nc) as tc:
        with (
            tc.tile_pool(name="consts", bufs=1) as cp,
            tc.tile_pool(name="io", bufs=1) as iop,
            tc.tile_pool(name="oh", bufs=1) as ohp,
            tc.tile_pool(name="acc", bufs=1, space="PSUM") as psa,
            tc.tile_pool(name="fin", bufs=1) as fp,
        ):
            # ---- constants ----
            iota32 = cp.tile([P, TT, C16], i32)
            nc.gpsimd.iota(
                iota32[:, :, :], pattern=[[0, TT], [1, C16]], base=0,
                channel_multiplier=0,
            )
            iota8 = cp.tile([P, TT, C16], f8)
            nc.vector.tensor_copy(iota8[:, :, :], iota32[:, :, :])
            tgt_sb = cp.tile([P, T], f8)
            nc.scalar.dma_start(out=tgt_sb[:, :], in_=tgtv[:, :])

            # ---- all ext tile DMAs up-front, alternating HWDGE rings.
            # One SBUF tile per DMA: disjoint writes into a single big
            # tile get serialized by whole-tile WAW tracking (measured:
            # 24.8us DMA span at 24% engine duty), separate tiles don't.
            exts = []
            for i, (t0, tt) in enumerate(splits):
                e = iop.tile([P, tt, DE], f8, name=f"ext{i}", tag=f"ext{i}")
                eng = nc.sync if i % 2 == 0 else nc.scalar
                eng.dma_start(out=e[:, :, :], in_=extv[:, t0 : t0 + tt, :])
                exts.append(e)

            ohs = [
                ohp.tile([P, tt, C16], f8, name=f"oh{i}", tag=f"oh{i}")
                for i, (t0, tt) in enumerate(splits)
            ]

            # PE accumulation bookkeeping
            accs = []
            for g in range(NGRP):
                a = psa.tile([32 * g + C16, DE], f32, name=f"accq{g}")
                accs.append(a[32 * g : 32 * g + C16, :])
            started = [False] * NGRP
            last_step = [-1] * NGRP
            for s in range(T // 2):
                last_step[s % NGRP] = s

            step = 0
            for i, (t0, tt) in enumerate(splits):
                # one-hot for this tile (vector engine, all-fp8)
                nc.vector.tensor_tensor(
                    out=ohs[i][:, :, :],
                    in0=tgt_sb[:, t0 : t0 + tt].unsqueeze(2).to_broadcast(
                        [P, tt, C16]
                    ),
                    in1=iota8[:, :tt, :],
                    op=mybir.AluOpType.is_equal,
                )
                # one-hot segment matmuls: fp8 DoubleRow contracts 2
                # chunks (256 points) per matmul, alternating PE column
                # groups so LDWEIGHTS hides under the other group's MM
                for t in range(0, tt, 2):
                    g = step % NGRP
                    nc.tensor.matmul(
                        accs[g],
                        lhsT=ohs[i][:, t : t + 2, :],
                        rhs=exts[i][:, t : t + 2, :],
                        start=not started[g],
                        stop=(step == last_step[g]),
                        perf_mode=mybir.MatmulPerfMode.DoubleRow,
                    )
                    started[g] = True
                    step += 1

            # ---- tail: stats to SBUF, single small out DMA ----
            out_sb = fp.tile([32 * (NGRP - 1) + C16, DE], f32)
            for g in range(NGRP):
                nc.vector.tensor_copy(
                    out_sb[32 * g : 32 * g + C16, :], accs[g]
                )
            nc.sync.dma_start(out=out_h[:, :], in_=out_sb[:, :])
    nc.finalize()
    return nc


# set by test.py to capture profile info
TRACE = False
LAST = {}


def _ensure_ntff_hook():
    """The agent image's antenv lacks axon_hooks; synthesize it so
    run_bass_kernel_spmd(trace=True) can profile. Best-effort."""
    import sys
    import types

    try:
        from antenv.axon_hooks import get_axon_ntff_profile_hook  # noqa: F401
        return
    except ImportError:
        pass
    try:
        from trn_agent_boot.trn_boot import _ntff_profile_via_ctypes

        hook = _ntff_profile_via_ctypes("/opt/axon/libaxon_pjrt.so")
        mod = types.ModuleType("antenv.axon_hooks")
        mod._hook = hook
        mod.get_axon_ntff_profile_hook = lambda: mod._hook
        mod.set_axon_ntff_profile_hook = lambda h: setattr(mod, "_hook", h)
        sys.modules["antenv.axon_hooks"] = mod
        import antenv

        antenv.axon_hooks = mod
    except Exception as e:  # degrade: no profile, run still works
        print(f"ntff hook injection failed: {e}")


def kernel(pred=None, target=None, feat=None, batch=None, centers=None):
    target = np.asarray(target)
    feat = np.asarray(feat, dtype=np.float32)
    batch = np.asarray(batch)
    centers = np.asarray(centers, dtype=np.float64)
    N = feat.shape[0]

    # shard at batch boundaries: core b <- batch b (batch is sorted)
    bounds = np.searchsorted(batch, np.arange(B + 1))
    sizes = np.diff(bounds)
    T = int(max((int(sizes.max()) + P - 1) // P, TT))
    T += T % 2  # DoubleRow matmuls consume point-pairs
    Npad = P * T

    feat8 = feat.astype(NP_F8)
    hsq8 = ((feat8.astype(np.float32) ** 2).sum(1) / SQ_SCALE).astype(NP_F8)
    in_maps = []
    for b in range(B):
        lo, hi = int(bounds[b]), int(bounds[b + 1])
        n = hi - lo
        ext = np.zeros((Npad, DE), dtype=NP_F8)
        ext[:n, :D] = feat8[lo:hi]
        ext[:n, D] = np.asarray(1.0, dtype=NP_F8)
        ext[:n, D + 1] = hsq8[lo:hi]
        tb = np.full((Npad,), C16, dtype=np.float32)
        tb[:n] = target[lo:hi]
        inv = tb == IGNORE
        if inv.any():
            tb[inv] = C16  # one-hot miss -> excluded everywhere
            ext[inv] = np.asarray(0.0, dtype=NP_F8)
        in_maps.append({"ext": ext, "tgt": tb.astype(NP_F8)})

    nc = build_nc(T)
    if TRACE:
        _ensure_ntff_hook()
    res = run_bass_kernel_spmd(nc, in_maps, list(range(N_CORES)), trace=TRACE)
    LAST["results"] = res

    # ---- host finale (tiny: 8 cores x [13, 66] stats) ----
    intra_sum = 0.0
    inter_sum = 0.0
    present_cnt = 0
    cn2 = (centers ** 2).sum(1)  # (13,)
    for b in range(B):
        o = np.asarray(res.results[b]["out"]).astype(np.float64)
        stats = o[0:C, :].copy()
        for g in range(1, NGRP):
            stats += o[32 * g : 32 * g + C, :]
        fsum = stats[:, :D]  # (13, 64)
        ccnt = stats[:, D]  # (13,)
        S = SQ_SCALE * stats[:, D + 1].sum()
        cnt_b = ccnt.sum()
        if cnt_b <= 0:
            continue
        present_cnt += 1
        # intra: S - 2 sum_c c.fsum + sum_c ccnt*||c||^2, / cnt
        tdot = float((centers * fsum).sum())
        utot = float((ccnt * cn2).sum())
        intra_sum += (S - 2.0 * tdot + utot) / cnt_b
        # inter: pairwise hinge on class means
        pres = ccnt > 0
        cm = fsum / np.maximum(ccnt, 1.0)[:, None]
        diff = cm[:, None, :] - cm[None, :, :]
        dd2 = (diff ** 2).sum(-1)
        eye = np.eye(C, dtype=bool)
        pm = pres[:, None] & pres[None, :] & ~eye
        dist = np.sqrt(np.where(pm, dd2, 1.0))
        terms = np.where(pm, np.maximum(MARGIN - dist, 0.0), 0.0)
        npairs = pm.sum()
        inter_sum += terms.sum() / max(npairs, 1)

    den = max(present_cnt, 1)
    loss = LOSS_W * (INTRA_W * intra_sum / den + INTER_W * inter_sum / den)
    return np.float32(loss)
